# revision 36
# baseline (speedup 1.0000x reference)
"""Trainium2 Bass kernel for nn_CausalSelfAttention_67894843015857.

Full inputs -> full output. Sharding: 8 cores = 2 batches x 4 head-groups
(4 heads each). Per core, on device (all matmul operands bf16, PSUM f32):
  - q,k projections (W stationary, x^T moving) -> qT/kT in [dim, t] layout
  - RoPE (host-precomputed cos/sin tables from token_index histogram cumsum)
    done full-width per head-pair via one DMA-built partition-rotated copy,
    then fork-channel row overwrites
  - v projection (x^T stationary) -> V in [t, dim] layout, scaled by
    exp(cumulative_scores)*padmask, with a ones-column appended (softmax denom)
  - attention with TRANSPOSED scores S^T[tk, tq] (no P-transpose needed),
    no-max-subtraction softmax (scores bounded ~2.6), causal masking via
    0/1 masks on the 4 diagonal blocks of each 512-wide tq chunk.
    Chunk-major over tq so the output projection for chunk c-1 interleaves
    into chunk c's attention (overlaps the out DMA with compute).
  - output projection -> per-core partial [T, C] stored bf16
Host: reduces the 4 per-batch partials and adds b_proj.
"""
import numpy as np
import ml_dtypes

import concourse.bacc as bacc
import concourse.mybir as mybir
import concourse.tile as tile
from concourse.bass_utils import run_bass_kernel_spmd

F32 = mybir.dt.float32
F32R = mybir.dt.float32r
BF16 = mybir.dt.bfloat16
AF = mybir.ActivationFunctionType
BF = ml_dtypes.bfloat16

P = 128
T = 2048
C = 1024
NKT = C // P          # 8 contraction tiles over the embedding dim
NT = T // P           # 16 t-tiles
SCALE = 0.125         # 1/sqrt(64)
NCHUNK = 4            # tq chunks of 512
CH = 512

_NC_CACHE = {}


def build_nc(has_vbias=False):
    if ("nc", has_vbias) in _NC_CACHE:
        return _NC_CACHE[("nc", has_vbias)]
    nc = bacc.Bacc("TRN2", target_bir_lowering=False, debug=False)

    def din(name, shape, dt=BF16):
        return nc.dram_tensor(name, shape, dt, kind="ExternalInput").ap()

    xt_d = din("xt", [C, T])            # x[b].T
    wqk_d = din("wqk", [C, 512])        # [q cols 256 | k cols 256] for this head group
    wv_d = din("wv", [C, 256])
    wp_d = din("wp", [256, C])
    bqk_d = din("bqk", [P, 4], F32)     # bias per col-tile m
    bv_d = din("bv", [1, 256])
    cos_d = din("cos2", [P, T])         # cos table (rows 63,127 zeroed)
    sin_d = din("sintab", [P, T])       # +sin table (rows 63,127 = 1.0)
    rsel_d = din("rsel", [P, P])        # signed rotate-half permutation (lhsT)
    fsel_d = din("fsel", [1, P])        # one-hot fork-row selector (lhsT)
    cs_d = din("csrow", [1, T])         # cumulative_scores[b] (fork-k rows)
    vs_d = din("vscale", [P, 16], F32)  # exp(cs)*pmbin, t-tiled columns
    oc_d = din("onec", [P, 16])         # pmbin, t-tiled columns
    dm_d = din("dmask", [P, P])         # upper-tri 0/1 mask: m[p,c] = c >= p
    ones_d = din("ones", [1, T])        # fork-q rows + bf16 bias-matmul lhsT
    onr_d = din("onesr", [1, 64], F32R)  # f32r lhsT for the norm broadcast
    out_d = nc.dram_tensor("outp", [T, C], BF16, kind="ExternalOutput").ap()

    with tile.TileContext(nc) as tc:
        with tc.tile_pool(name="const", bufs=1) as pc, \
             tc.tile_pool(name="persist", bufs=1) as pp:
            bqk_sb = pc.tile([P, 4], F32, name="bqk_sb")
            bv_sb = pc.tile([1, 256], BF16, name="bv_sb")
            ones_sb = pc.tile([1, T], BF16, name="ones_sb")
            onr_sb = pc.tile([1, 64], F32R, name="onr_sb")
            cs_sb = pc.tile([1, T], BF16, name="cs_sb")
            vs_sb = pc.tile([P, 16], F32, name="vs_sb")
            oc_sb = pc.tile([P, 16], BF16, name="oc_sb")
            dm_sb = pc.tile([P, P], BF16, name="dm_sb")
            rsel_sb = pc.tile([P, P], BF16, name="rsel_sb")
            fsel_sb = pc.tile([1, P], BF16, name="fsel_sb")

            qk_t = [pp.tile([P, T], BF16, name=f"qkt{m}") for m in range(4)]
            vv = pp.tile([P, NT * 260], BF16, name="vv")
            yt = [pp.tile([P, T], BF16, name=f"yt{i}") for i in range(2)]
            wp_sb = pp.tile([P, 2 * C], BF16, name="wp_sb")

            pat = tc.alloc_tile_pool(name="attn_sb", bufs=5)
            pnm = tc.alloc_tile_pool(name="norm_sb", bufs=2)
            pmid = tc.alloc_tile_pool(name="mid", bufs=1)
            with tc.tile_pool(name="load", bufs=1) as pl:
                xt = pl.tile([P, NKT * T], BF16, name="xt_sb")
                wqk = pl.tile([P, NKT * 512], BF16, name="wqk_sb")
                # k=0 weights + x chunks first (earliest matmul start)
                nc.sync.dma_start(wqk[:, 0:512], wqk_d[0:P, :])
                for n in range(4):
                    nc.sync.dma_start(xt[:, n * CH:(n + 1) * CH],
                                      xt_d[0:P, n * CH:(n + 1) * CH])
                for k in range(1, NKT):
                    nc.sync.dma_start(wqk[:, k * 512:(k + 1) * 512],
                                      wqk_d[k * P:(k + 1) * P, :])
                    nc.sync.dma_start(xt[:, k * T:(k + 1) * T],
                                      xt_d[k * P:(k + 1) * P, :])
                cos_sb = pmid.tile([P, T], BF16, name="cos_sb")
                nc.sync.dma_start(cos_sb[:], cos_d[:])
                sin_sb = pmid.tile([P, T], BF16, name="sin_sb")
                nc.sync.dma_start(sin_sb[:], sin_d[:])
                nc.sync.dma_start(rsel_sb[:], rsel_d[:])
                nc.sync.dma_start(fsel_sb[:], fsel_d[:])
                nc.sync.dma_start(bqk_sb[:], bqk_d[:])
                nc.sync.dma_start(bv_sb[:], bv_d[:])
                nc.sync.dma_start(ones_sb[:], ones_d[:])
                nc.sync.dma_start(onr_sb[:], onr_d[:])
                nc.sync.dma_start(cs_sb[:], cs_d[:])
                nc.sync.dma_start(vs_sb[:], vs_d[:])
                nc.sync.dma_start(oc_sb[:], oc_d[:])
                nc.sync.dma_start(dm_sb[:], dm_d[:])
                wv = pl.tile([P, NKT * 256], BF16, name="wv_sb")
                nc.sync.dma_start(
                    wv[:].rearrange("p (k c) -> p k c", k=NKT),
                    wv_d[:].rearrange("(k p) c -> p k c", k=NKT))
                nc.sync.dma_start(
                    wp_sb[:].rearrange("p (k c) -> p k c", k=2),
                    wp_d[:].rearrange("(k p) c -> p k c", k=2))

                def qk_mm(ms, qkps, tagbase=0):
                    """q,k projections for col-tiles `ms`, k-outer so PE rides
                    the incoming xt DMA stream; then psum->sbuf bias copies."""
                    pss = {(m, n): qkps.tile([P, CH], F32, name=f"qkps{m}_{n}",
                                             tag=f"a{tagbase + 4 * ms.index(m) + n}")
                           for m in ms for n in range(4)}
                    for k in range(NKT):
                        for m in ms:
                            for n in range(4):
                                nc.tensor.matmul(
                                    pss[m, n][:],
                                    lhsT=wqk[:, k * 512 + m * P: k * 512 + (m + 1) * P],
                                    rhs=xt[:, k * T + n * CH: k * T + (n + 1) * CH],
                                    start=(k == 0), stop=(k == NKT - 1))
                    # psum->sbuf copies split ACT/DVE so the bridge into the
                    # next PSUM users drains twice as fast
                    for n in range(4):
                        for m in ms:
                            ns = slice(n * CH, (n + 1) * CH)
                            if (n + ms.index(m)) % 2 == 0:
                                nc.scalar.activation(qk_t[m][:, ns], pss[m, n][:],
                                                     AF.Identity,
                                                     bias=bqk_sb[:, m:m + 1])
                            else:
                                nc.vector.tensor_scalar_add(
                                    qk_t[m][:, ns], pss[m, n][:],
                                    bqk_sb[:, m:m + 1])

                def rope_group(m, ps, tagbase):
                    """Chunked in-place RoPE on qk_t[m]: the rotate-half copy
                    comes from a signed-permutation matmul (plus a K=1 matmul
                    injecting the fork rows: cos rows 63,127 are zeroed and
                    sin rows are 1.0 there, so rope yields the fork values)."""
                    src = ones_sb if m < 2 else cs_sb
                    for n in range(4):
                        ns = slice(n * CH, (n + 1) * CH)
                        qs = ps.tile([P, CH], F32, name=f"qs{m}_{n}",
                                     tag=f"a{tagbase + n}")
                        nc.tensor.matmul(qs[:], lhsT=rsel_sb[:],
                                         rhs=qk_t[m][:, ns],
                                         start=True, stop=False)
                        nc.tensor.matmul(qs[:], lhsT=fsel_sb[0:1, :],
                                         rhs=src[0:1, ns],
                                         start=False, stop=True)
                        qsh = pmid.tile([P, CH], BF16, name=f"qsh{m}_{n}",
                                        tag="qsh", bufs=2)
                        nc.vector.tensor_mul(qk_t[m][:, ns], qk_t[m][:, ns],
                                             cos_sb[:, ns])
                        nc.vector.tensor_mul(qsh[:], qs[:], sin_sb[:, ns])
                        nc.vector.tensor_add(qk_t[m][:, ns], qk_t[m][:, ns],
                                             qsh[:])

                def v_group(mt0, mt1, vpsp):
                    """v projection: out[t, vcol] = xT_tile.T @ wv; build V''."""
                    for mt in range(mt0, mt1):
                        vps = vpsp.tile([P, 256], F32, name=f"vps{mt}",
                                        tag=f"a{mt % 4}")
                        for k in range(NKT):
                            nc.tensor.matmul(
                                vps[:],
                                lhsT=xt[:, k * T + mt * P: k * T + (mt + 1) * P],
                                rhs=wv[:, k * 256:(k + 1) * 256],
                                start=(k == 0),
                                stop=(not has_vbias and k == NKT - 1))
                        if has_vbias:
                            nc.tensor.matmul(vps[:], lhsT=ones_sb[0:1, 0:P],
                                             rhs=bv_sb[0:1, :], start=False,
                                             stop=True)
                        vvs = vv[:, mt * 260:(mt + 1) * 260].rearrange(
                            "p (h x) -> p h x", x=65)
                        nc.vector.tensor_scalar_mul(
                            vvs[:, :, 0:64],
                            vps[:].rearrange("p (h x) -> p h x", x=64),
                            vs_sb[:, mt:mt + 1])
                        nc.vector.tensor_copy(
                            vvs[:, :, 64:65],
                            oc_sb[:, mt:mt + 1, None].to_broadcast((P, 4, 1)))

                with nc.named_scope("proj"), \
                     tc.tile_pool(name="ps1", bufs=1, space="PSUM") as ps1:
                    qk_mm((0, 2), ps1)     # q heads 0,1 + k heads 0,1
                    rope_group(0, ps1, 0)
                    rope_group(2, ps1, 4)
                    v_group(0, 4, ps1)
                    qk_mm((1,), ps1, 4)    # q heads 2,3
                    rope_group(1, ps1, 4)
                    qk_mm((3,), ps1, 4)    # k heads 2,3
                    rope_group(3, ps1, 4)
                    v_group(4, 16, ps1)

            # load pool released: xt/wqk/wv space reusable
            def norm_chunk(h, cch, yps, pyps, psps):
                # normalize: y = num / den  (den = ones-column row 64)
                ti = h // 2
                ro = 64 * (h % 2)
                recip = pnm.tile([1, CH], F32R, name=f"rc_{h}_{cch}", tag="rc")
                with nc.allow_low_precision(reason="f32r recip feeds f32r bcast matmul"):
                    nc.vector.reciprocal(recip[0:1, :], yps[64:65, :])
                # broadcast along partitions via K=1 ones matmul
                bps = psps.tile([64, CH], F32, name=f"bp_{h}_{cch}", tag="scr")
                nc.tensor.matmul(bps[:], lhsT=onr_sb[0:1, 0:64],
                                 rhs=recip[0:1, :], start=True, stop=True)
                bsb = pnm.tile([64, CH], F32, name=f"bs_{h}_{cch}", tag="bs")
                if (h + cch) % 2 == 0:
                    nc.vector.tensor_copy(bsb[:], bps[:])
                else:
                    nc.scalar.copy(bsb[:], bps[:])
                nc.vector.tensor_mul(
                    yt[ti][ro:ro + 64, cch * CH:(cch + 1) * CH],
                    yps[0:64, :], bsb[:])

            norm_state = {"pending": None}

            def attn_chunk(h, cch, psps, pyps):
                ti = h // 2
                ro = 64 * (h % 2)
                qt = qk_t[ti]
                kt = qk_t[2 + ti]
                nik = 4 * (cch + 1)
                yps = pyps.tile([65, CH], F32, name=f"yps_{h}_{cch}", tag="yps")
                for p2 in range(nik // 2):
                    # an ik pair shares one 2-bank PSUM tile so full
                    # pairs need only a single wide exp
                    spw = psps.tile([P, 2 * CH], F32,
                                    name=f"spw_{h}_{cch}_{p2}", tag="sps")
                    pt = pat.tile([P, 2 * CH], BF16,
                                  name=f"pt_{h}_{cch}_{p2}", tag="pt")
                    iks = (2 * p2, 2 * p2 + 1)
                    los = [max(ik - 4 * cch, 0) * P for ik in iks]
                    for ii, ik in enumerate(iks):
                        lo = los[ii]
                        nc.tensor.matmul(
                            spw[:, ii * CH + lo:(ii + 1) * CH],
                            lhsT=kt[ro:ro + 64, ik * P:(ik + 1) * P],
                            rhs=qt[ro:ro + 64, cch * CH + lo:(cch + 1) * CH],
                            start=True, stop=True)
                    if los[0] == 0 and los[1] == 0:
                        nc.scalar.activation(pt[:], spw[:], AF.Exp,
                                             scale=SCALE)
                    else:
                        for ii, ik in enumerate(iks):
                            lo = los[ii]
                            nc.scalar.activation(
                                pt[:, ii * CH + lo:(ii + 1) * CH],
                                spw[:, ii * CH + lo:(ii + 1) * CH],
                                AF.Exp, scale=SCALE)
                    for ii, ik in enumerate(iks):
                        lo = los[ii]
                        if ik - 4 * cch >= 0:
                            # triangular mask on the diagonal block
                            nc.vector.tensor_mul(
                                pt[:, ii * CH + lo: ii * CH + lo + P],
                                pt[:, ii * CH + lo: ii * CH + lo + P],
                                dm_sb[:])
                        nc.tensor.matmul(
                            yps[:, lo:CH],
                            lhsT=vv[:, ik * 260 + h * 65: ik * 260 + h * 65 + 65],
                            rhs=pt[:, ii * CH + lo:(ii + 1) * CH],
                            start=(ik == 0), stop=(ik == nik - 1))
                    if p2 == 0 and norm_state["pending"] is not None:
                        # previous chunk's norm, deep in this chunk's
                        # pipeline so it never stalls PE/ACT
                        norm_chunk(*norm_state["pending"], pyps, psps)
                        norm_state["pending"] = None
                norm_state["pending"] = (h, cch, yps)

            def outproj_chunk(cch, psps, pout):
                for mt in range(4 * cch, 4 * cch + 4):
                    osb = pout.tile([P, C], BF16, name=f"osb{mt}", tag="osb")
                    for n in range(2):
                        pps = psps.tile([P, CH], F32, name=f"pps{mt}_{n}",
                                        tag="scr")
                        for kk in range(2):
                            nc.tensor.matmul(
                                pps[:],
                                lhsT=yt[kk][:, mt * P:(mt + 1) * P],
                                rhs=wp_sb[:, kk * C + n * CH: kk * C + (n + 1) * CH],
                                start=(kk == 0), stop=(kk == 1))
                        if n == 0:
                            nc.scalar.copy(osb[:, 0:CH], pps[:])
                        else:
                            nc.vector.tensor_copy(osb[:, CH:C], pps[:])
                    nc.sync.dma_start(out_d[mt * P:(mt + 1) * P, :], osb[:])

            with nc.named_scope("attn"), \
                 tc.tile_pool(name="ps2", bufs=2, space="PSUM") as psps, \
                 tc.tile_pool(name="out_sb", bufs=3) as pout:
                # h2/h3 chunks deferred so their RoPE (critical path out of
                # the projection phase) never gates PE; out-proj for chunk c
                # goes in once all 4 of its norms have flushed
                sched = [(0, 0), (1, 0), (0, 1), (1, 1), (2, 0), (3, 0),
                         (2, 1), (3, 1), "o0", (0, 2), (1, 2), "o1",
                         (2, 2), (3, 2), (0, 3), (1, 3), "o2", (2, 3), (3, 3)]
                for item in sched:
                    if isinstance(item, str):
                        outproj_chunk(int(item[1]), psps, pout)
                    else:
                        attn_chunk(item[0], item[1], psps, psps)
                norm_chunk(*norm_state["pending"], psps, psps)
                outproj_chunk(3, psps, pout)

            pmid.release()
            pnm.release()
            pat.release()
    nc.compile()
    _NC_CACHE[("nc", has_vbias)] = nc
    return nc


def make_in_maps(x, cumulative_scores, padding_mask, W_attn, b_attn, W_proj,
                 b_proj, token_index):
    x = np.asarray(x, dtype=np.float32)
    cs = np.asarray(cumulative_scores, dtype=np.float32)
    pm = np.asarray(padding_mask, dtype=np.float32)
    Wa = np.asarray(W_attn, dtype=np.float32)
    ba = np.asarray(b_attn, dtype=np.float32)
    Wp = np.asarray(W_proj, dtype=np.float32)
    tok = np.asarray(token_index).astype(np.int64)
    B = x.shape[0]

    # single upper-triangular 0/1 mask for diagonal blocks: m[p,c] = c >= p
    dmask = (np.arange(P)[None, :] >= np.arange(P)[:, None]).astype(BF)
    ones_row = np.ones((1, T), BF)
    ones_r = np.ones((1, 64), np.float32)
    # signed rotate-half permutation as matmul lhsT: rsel[p, d] = s(d) where
    # p = rot(d); fork columns 63/127 zeroed (fork rows come from fsel)
    rsel = np.zeros((P, P), np.float32)
    for blk in (0, 64):
        for j in range(32):
            rsel[blk + j + 32, blk + j] = -1.0      # dst j < 32: -x[j+32]
        for j in range(32, 63):
            rsel[blk + j - 32, blk + j] = 1.0       # dst j >= 32: +x[j-32]
        rsel[blk + 31, blk + 63] = 0.0
    fsel = np.zeros((1, P), np.float32)
    fsel[0, 63] = 1.0
    fsel[0, 127] = 1.0

    per_batch = []
    for b in range(B):
        counts = np.bincount(tok[b], minlength=T).astype(np.float32)
        with np.errstate(divide="ignore"):
            invc = (1.0 / counts).astype(np.float32)
        partial = np.cumsum(invc[tok[b]], dtype=np.float32)
        invf = (1.0 / (10000.0 ** (np.arange(0, 64, 2, dtype=np.float32) / 64.0))
                ).astype(np.float32)
        ang = partial[:, None].astype(np.float32) * invf[None, :]
        cos32 = np.cos(ang).T.astype(np.float32)
        sin32 = np.sin(ang).T.astype(np.float32)
        # fork rows folded into the tables: cos rows 63/127 -> 0 and sin
        # rows -> 1, so rope emits the fork values injected by the fsel matmul
        cos128 = np.tile(cos32, (4, 1))
        cos128[63, :] = 0.0
        cos128[127, :] = 0.0
        sin128 = np.tile(sin32, (4, 1))
        sin128[63, :] = 1.0
        sin128[127, :] = 1.0
        pmg = np.take_along_axis(pm[b], tok[b], axis=0).astype(np.float32)
        pmbin = (pmg != 0).astype(np.float32)
        vscale = (np.exp(cs[b]).astype(np.float32) * pmbin).astype(np.float32)
        per_batch.append({
            "xt": np.ascontiguousarray(x[b].T).astype(BF),
            "cos2": np.ascontiguousarray(cos128).astype(BF),
            "sintab": np.ascontiguousarray(sin128).astype(BF),
            "csrow": np.ascontiguousarray(cs[b][None, :]).astype(BF),
            "vscale": np.ascontiguousarray(vscale.reshape(NT, P).T),
            "onec": np.ascontiguousarray(pmbin.reshape(NT, P).T).astype(BF),
        })

    in_maps = []
    for core in range(8):
        b = core // 4
        g = core % 4
        qc = slice(g * 256, (g + 1) * 256)
        kc = slice(C + g * 256, C + (g + 1) * 256)
        vc = slice(2 * C + g * 256, 2 * C + (g + 1) * 256)
        wqk = np.ascontiguousarray(
            np.concatenate([Wa[:, qc], Wa[:, kc]], axis=1)).astype(BF)
        bqk_flat = np.concatenate([ba[qc], ba[kc]])          # [512]
        bqk = np.ascontiguousarray(bqk_flat.reshape(4, P).T)
        in_maps.append({
            **per_batch[b],
            "wqk": wqk,
            "wv": np.ascontiguousarray(Wa[:, vc]).astype(BF),
            "wp": np.ascontiguousarray(Wp[g * 256:(g + 1) * 256, :]).astype(BF),
            "bqk": bqk,
            "bv": np.ascontiguousarray(ba[vc][None, :]).astype(BF),
            "dmask": dmask,
            "ones": ones_row,
            "onesr": ones_r,
            "rsel": rsel.astype(BF),
            "fsel": fsel.astype(BF),
        })
    return in_maps


def kernel(x, cumulative_scores, padding_mask, W_attn, b_attn, W_proj, b_proj,
           token_index, _results_hook=None):
    nc = build_nc(has_vbias=bool(np.any(np.asarray(b_attn)[2 * C:])))
    in_maps = make_in_maps(x, cumulative_scores, padding_mask, W_attn, b_attn,
                           W_proj, b_proj, token_index)
    res = run_bass_kernel_spmd(nc, in_maps, list(range(8)))
    if _results_hook is not None:
        _results_hook(res)
    bp = np.asarray(b_proj, dtype=np.float32)
    B = np.asarray(x).shape[0]
    out = np.zeros((B, T, C), np.float32)
    for b in range(B):
        acc = np.zeros((T, C), np.float32)
        for g in range(4):
            acc += np.asarray(res.results[b * 4 + g]["outp"]).astype(np.float32)
        out[b] = acc + bp[None, :]
    return out


# revision 37
# speedup vs baseline: 1.0170x; 1.0170x over previous
"""Trainium2 Bass kernel for nn_CausalSelfAttention_67894843015857.

Full inputs -> full output. Sharding: 8 cores = 2 batches x 4 head-groups
(4 heads each). Per core, on device (all matmul operands bf16, PSUM f32):
  - q,k projections (W stationary, x^T moving) -> qT/kT in [dim, t] layout
  - RoPE (host-precomputed cos/sin tables from token_index histogram cumsum)
    done full-width per head-pair via one DMA-built partition-rotated copy,
    then fork-channel row overwrites
  - v projection (x^T stationary) -> V in [t, dim] layout, scaled by
    exp(cumulative_scores)*padmask, with a ones-column appended (softmax denom)
  - attention with TRANSPOSED scores S^T[tk, tq] (no P-transpose needed),
    no-max-subtraction softmax (scores bounded ~2.6), causal masking via
    0/1 masks on the 4 diagonal blocks of each 512-wide tq chunk.
    Chunk-major over tq so the output projection for chunk c-1 interleaves
    into chunk c's attention (overlaps the out DMA with compute).
  - output projection -> per-core partial [T, C] stored bf16
Host: reduces the 4 per-batch partials and adds b_proj.
"""
import numpy as np
import ml_dtypes

import concourse.bacc as bacc
import concourse.mybir as mybir
import concourse.tile as tile
from concourse.bass_utils import run_bass_kernel_spmd

F32 = mybir.dt.float32
F32R = mybir.dt.float32r
BF16 = mybir.dt.bfloat16
AF = mybir.ActivationFunctionType
BF = ml_dtypes.bfloat16

P = 128
T = 2048
C = 1024
NKT = C // P          # 8 contraction tiles over the embedding dim
NT = T // P           # 16 t-tiles
SCALE = 0.125         # 1/sqrt(64)
NCHUNK = 4            # tq chunks of 512
CH = 512

_NC_CACHE = {}


def build_nc(has_vbias=False):
    if ("nc", has_vbias) in _NC_CACHE:
        return _NC_CACHE[("nc", has_vbias)]
    nc = bacc.Bacc("TRN2", target_bir_lowering=False, debug=False)

    def din(name, shape, dt=BF16):
        return nc.dram_tensor(name, shape, dt, kind="ExternalInput").ap()

    xt_d = din("xt", [C, T])            # x[b].T
    wqk_d = din("wqk", [C, 512])        # [q cols 256 | k cols 256] for this head group
    wv_d = din("wv", [C, 256])
    wp_d = din("wp", [256, C])
    bqk_d = din("bqk", [P, 4], F32)     # bias per col-tile m
    bv_d = din("bv", [1, 256])
    cos_d = din("cos2", [P, T])         # cos table (rows 63,127 zeroed)
    sin_d = din("sintab", [P, T])       # +sin table (rows 63,127 = 1.0)
    rsel_d = din("rsel", [P, P])        # signed rotate-half permutation (lhsT)
    fsel_d = din("fsel", [1, P])        # one-hot fork-row selector (lhsT)
    cs_d = din("csrow", [1, T])         # cumulative_scores[b] (fork-k rows)
    vs_d = din("vscale", [P, 16], F32)  # exp(cs)*pmbin, t-tiled columns
    oc_d = din("onec", [P, 16])         # pmbin, t-tiled columns
    dm_d = din("dmask", [P, P])         # upper-tri 0/1 mask: m[p,c] = c >= p
    ones_d = din("ones", [1, T])        # fork-q rows + bf16 bias-matmul lhsT
    onr_d = din("onesr", [1, 64], F32R)  # f32r lhsT for the norm broadcast
    out_d = nc.dram_tensor("outp", [T, C], BF16, kind="ExternalOutput").ap()

    with tile.TileContext(nc) as tc:
        with tc.tile_pool(name="const", bufs=1) as pc, \
             tc.tile_pool(name="persist", bufs=1) as pp:
            bqk_sb = pc.tile([P, 4], F32, name="bqk_sb")
            bv_sb = pc.tile([1, 256], BF16, name="bv_sb")
            ones_sb = pc.tile([1, T], BF16, name="ones_sb")
            onr_sb = pc.tile([1, 64], F32R, name="onr_sb")
            cs_sb = pc.tile([1, T], BF16, name="cs_sb")
            vs_sb = pc.tile([P, 16], F32, name="vs_sb")
            oc_sb = pc.tile([P, 16], BF16, name="oc_sb")
            dm_sb = pc.tile([P, P], BF16, name="dm_sb")
            rsel_sb = pc.tile([P, P], BF16, name="rsel_sb")
            fsel_sb = pc.tile([1, P], BF16, name="fsel_sb")

            qk_t = [pp.tile([P, T], BF16, name=f"qkt{m}") for m in range(4)]
            vv = pp.tile([P, NT * 260], BF16, name="vv")
            yt = [pp.tile([P, T], BF16, name=f"yt{i}") for i in range(2)]
            wp_sb = pp.tile([P, 2 * C], BF16, name="wp_sb")

            pat = tc.alloc_tile_pool(name="attn_sb", bufs=5)
            pnm = tc.alloc_tile_pool(name="norm_sb", bufs=2)
            pmid = tc.alloc_tile_pool(name="mid", bufs=1)
            with tc.tile_pool(name="load", bufs=1) as pl:
                xt = pl.tile([P, NKT * T], BF16, name="xt_sb")
                wqk = pl.tile([P, NKT * 512], BF16, name="wqk_sb")
                # k=0 weights + x chunks first (earliest matmul start)
                nc.sync.dma_start(wqk[:, 0:512], wqk_d[0:P, :])
                for n in range(4):
                    nc.sync.dma_start(xt[:, n * CH:(n + 1) * CH],
                                      xt_d[0:P, n * CH:(n + 1) * CH])
                for k in range(1, NKT):
                    nc.sync.dma_start(wqk[:, k * 512:(k + 1) * 512],
                                      wqk_d[k * P:(k + 1) * P, :])
                    nc.sync.dma_start(xt[:, k * T:(k + 1) * T],
                                      xt_d[k * P:(k + 1) * P, :])
                cos_sb = pmid.tile([P, T], BF16, name="cos_sb")
                nc.sync.dma_start(cos_sb[:], cos_d[:])
                sin_sb = pmid.tile([P, T], BF16, name="sin_sb")
                nc.sync.dma_start(sin_sb[:], sin_d[:])
                nc.sync.dma_start(rsel_sb[:], rsel_d[:])
                nc.sync.dma_start(fsel_sb[:], fsel_d[:])
                nc.sync.dma_start(bqk_sb[:], bqk_d[:])
                nc.sync.dma_start(bv_sb[:], bv_d[:])
                nc.sync.dma_start(ones_sb[:], ones_d[:])
                nc.sync.dma_start(onr_sb[:], onr_d[:])
                nc.sync.dma_start(cs_sb[:], cs_d[:])
                nc.sync.dma_start(vs_sb[:], vs_d[:])
                nc.sync.dma_start(oc_sb[:], oc_d[:])
                nc.sync.dma_start(dm_sb[:], dm_d[:])
                wv = pl.tile([P, NKT * 256], BF16, name="wv_sb")
                nc.sync.dma_start(
                    wv[:].rearrange("p (k c) -> p k c", k=NKT),
                    wv_d[:].rearrange("(k p) c -> p k c", k=NKT))
                nc.sync.dma_start(
                    wp_sb[:].rearrange("p (k c) -> p k c", k=2),
                    wp_d[:].rearrange("(k p) c -> p k c", k=2))

                def qk_mm(ms, qkps, tagbase=0):
                    """q,k projections for col-tiles `ms`, k-outer so PE rides
                    the incoming xt DMA stream; then psum->sbuf bias copies."""
                    pss = {(m, n): qkps.tile([P, CH], F32, name=f"qkps{m}_{n}",
                                             tag=f"a{tagbase + 4 * ms.index(m) + n}")
                           for m in ms for n in range(4)}
                    for k in range(NKT):
                        for m in ms:
                            for n in range(4):
                                nc.tensor.matmul(
                                    pss[m, n][:],
                                    lhsT=wqk[:, k * 512 + m * P: k * 512 + (m + 1) * P],
                                    rhs=xt[:, k * T + n * CH: k * T + (n + 1) * CH],
                                    start=(k == 0), stop=(k == NKT - 1))
                    # psum->sbuf copies split ACT/DVE so the bridge into the
                    # next PSUM users drains twice as fast
                    for n in range(4):
                        for m in ms:
                            ns = slice(n * CH, (n + 1) * CH)
                            if (n + ms.index(m)) % 2 == 0:
                                nc.scalar.activation(qk_t[m][:, ns], pss[m, n][:],
                                                     AF.Identity,
                                                     bias=bqk_sb[:, m:m + 1])
                            else:
                                nc.vector.tensor_scalar_add(
                                    qk_t[m][:, ns], pss[m, n][:],
                                    bqk_sb[:, m:m + 1])

                def rope_group(m, ps, tagbase):
                    """Chunked in-place RoPE on qk_t[m]: the rotate-half copy
                    comes from a signed-permutation matmul (plus a K=1 matmul
                    injecting the fork rows: cos rows 63,127 are zeroed and
                    sin rows are 1.0 there, so rope yields the fork values)."""
                    src = ones_sb if m < 2 else cs_sb
                    for n in range(4):
                        ns = slice(n * CH, (n + 1) * CH)
                        qs = ps.tile([P, CH], F32, name=f"qs{m}_{n}",
                                     tag=f"a{tagbase + n}")
                        nc.tensor.matmul(qs[:], lhsT=rsel_sb[:],
                                         rhs=qk_t[m][:, ns],
                                         start=True, stop=False)
                        nc.tensor.matmul(qs[:], lhsT=fsel_sb[0:1, :],
                                         rhs=src[0:1, ns],
                                         start=False, stop=True)
                        qsh = pmid.tile([P, CH], BF16, name=f"qsh{m}_{n}",
                                        tag="qsh", bufs=2)
                        nc.vector.tensor_mul(qk_t[m][:, ns], qk_t[m][:, ns],
                                             cos_sb[:, ns])
                        nc.vector.tensor_mul(qsh[:], qs[:], sin_sb[:, ns])
                        nc.vector.tensor_add(qk_t[m][:, ns], qk_t[m][:, ns],
                                             qsh[:])

                def v_group(mt0, mt1, vpsp):
                    """v projection: out[t, vcol] = xT_tile.T @ wv; build V''."""
                    for mt in range(mt0, mt1):
                        vps = vpsp.tile([P, 256], F32, name=f"vps{mt}",
                                        tag=f"a{mt % 4}")
                        for k in range(NKT):
                            nc.tensor.matmul(
                                vps[:],
                                lhsT=xt[:, k * T + mt * P: k * T + (mt + 1) * P],
                                rhs=wv[:, k * 256:(k + 1) * 256],
                                start=(k == 0),
                                stop=(not has_vbias and k == NKT - 1))
                        if has_vbias:
                            nc.tensor.matmul(vps[:], lhsT=ones_sb[0:1, 0:P],
                                             rhs=bv_sb[0:1, :], start=False,
                                             stop=True)
                        vvs = vv[:, mt * 260:(mt + 1) * 260].rearrange(
                            "p (h x) -> p h x", x=65)
                        nc.vector.tensor_scalar_mul(
                            vvs[:, :, 0:64],
                            vps[:].rearrange("p (h x) -> p h x", x=64),
                            vs_sb[:, mt:mt + 1])
                        nc.vector.tensor_copy(
                            vvs[:, :, 64:65],
                            oc_sb[:, mt:mt + 1, None].to_broadcast((P, 4, 1)))

                with nc.named_scope("proj"), \
                     tc.tile_pool(name="ps1", bufs=1, space="PSUM") as ps1:
                    qk_mm((0, 2), ps1)     # q heads 0,1 + k heads 0,1
                    rope_group(0, ps1, 0)
                    rope_group(2, ps1, 4)
                    v_group(0, 4, ps1)
                    qk_mm((1,), ps1, 4)    # q heads 2,3
                    rope_group(1, ps1, 4)
                    qk_mm((3,), ps1, 4)    # k heads 2,3
                    rope_group(3, ps1, 4)
                    v_group(4, 16, ps1)

            # load pool released: xt/wqk/wv space reusable
            def norm_chunk(h, cch, yps, pyps, psps):
                # normalize: y = num / den  (den = ones-column row 64)
                ti = h // 2
                ro = 64 * (h % 2)
                recip = pnm.tile([1, CH], F32R, name=f"rc_{h}_{cch}", tag="rc")
                with nc.allow_low_precision(reason="f32r recip feeds f32r bcast matmul"):
                    nc.vector.reciprocal(recip[0:1, :], yps[64:65, :])
                # broadcast along partitions via K=1 ones matmul
                bps = psps.tile([64, CH], F32, name=f"bp_{h}_{cch}", tag="scr")
                nc.tensor.matmul(bps[:], lhsT=onr_sb[0:1, 0:64],
                                 rhs=recip[0:1, :], start=True, stop=True)
                bsb = pnm.tile([64, CH], F32, name=f"bs_{h}_{cch}", tag="bs")
                nc.vector.tensor_copy(bsb[:], bps[:])
                nc.vector.tensor_mul(
                    yt[ti][ro:ro + 64, cch * CH:(cch + 1) * CH],
                    yps[0:64, :], bsb[:])

            norm_state = {"pending": None}

            def attn_chunk(h, cch, psps, pyps):
                ti = h // 2
                ro = 64 * (h % 2)
                qt = qk_t[ti]
                kt = qk_t[2 + ti]
                nik = 4 * (cch + 1)
                yps = pyps.tile([65, CH], F32, name=f"yps_{h}_{cch}", tag="yps")
                for p2 in range(nik // 2):
                    # an ik pair shares one 2-bank PSUM tile so full
                    # pairs need only a single wide exp
                    spw = psps.tile([P, 2 * CH], F32,
                                    name=f"spw_{h}_{cch}_{p2}", tag="sps")
                    pt = pat.tile([P, 2 * CH], BF16,
                                  name=f"pt_{h}_{cch}_{p2}", tag="pt")
                    iks = (2 * p2, 2 * p2 + 1)
                    los = [max(ik - 4 * cch, 0) * P for ik in iks]
                    for ii, ik in enumerate(iks):
                        lo = los[ii]
                        nc.tensor.matmul(
                            spw[:, ii * CH + lo:(ii + 1) * CH],
                            lhsT=kt[ro:ro + 64, ik * P:(ik + 1) * P],
                            rhs=qt[ro:ro + 64, cch * CH + lo:(cch + 1) * CH],
                            start=True, stop=True)
                    if los[0] == 0 and los[1] == 0:
                        nc.scalar.activation(pt[:], spw[:], AF.Exp,
                                             scale=SCALE)
                    else:
                        for ii, ik in enumerate(iks):
                            lo = los[ii]
                            nc.scalar.activation(
                                pt[:, ii * CH + lo:(ii + 1) * CH],
                                spw[:, ii * CH + lo:(ii + 1) * CH],
                                AF.Exp, scale=SCALE)
                    for ii, ik in enumerate(iks):
                        lo = los[ii]
                        if ik - 4 * cch >= 0:
                            # triangular mask on the diagonal block
                            nc.vector.tensor_mul(
                                pt[:, ii * CH + lo: ii * CH + lo + P],
                                pt[:, ii * CH + lo: ii * CH + lo + P],
                                dm_sb[:])
                        nc.tensor.matmul(
                            yps[:, lo:CH],
                            lhsT=vv[:, ik * 260 + h * 65: ik * 260 + h * 65 + 65],
                            rhs=pt[:, ii * CH + lo:(ii + 1) * CH],
                            start=(ik == 0), stop=(ik == nik - 1))
                    if p2 == 0 and norm_state["pending"] is not None:
                        # previous chunk's norm, deep in this chunk's
                        # pipeline so it never stalls PE/ACT
                        norm_chunk(*norm_state["pending"], pyps, psps)
                        norm_state["pending"] = None
                norm_state["pending"] = (h, cch, yps)

            def outproj_chunk(cch, psps, pout):
                for mt in range(4 * cch, 4 * cch + 4):
                    osb = pout.tile([P, C], BF16, name=f"osb{mt}", tag="osb")
                    for n in range(2):
                        pps = psps.tile([P, CH], F32, name=f"pps{mt}_{n}",
                                        tag="scr")
                        for kk in range(2):
                            nc.tensor.matmul(
                                pps[:],
                                lhsT=yt[kk][:, mt * P:(mt + 1) * P],
                                rhs=wp_sb[:, kk * C + n * CH: kk * C + (n + 1) * CH],
                                start=(kk == 0), stop=(kk == 1))
                        if n == 0:
                            nc.scalar.copy(osb[:, 0:CH], pps[:])
                        else:
                            nc.vector.tensor_copy(osb[:, CH:C], pps[:])
                    nc.sync.dma_start(out_d[mt * P:(mt + 1) * P, :], osb[:])

            with nc.named_scope("attn"), \
                 tc.tile_pool(name="ps2", bufs=2, space="PSUM") as psps, \
                 tc.tile_pool(name="out_sb", bufs=3) as pout:
                # h2/h3 chunks deferred so their RoPE (critical path out of
                # the projection phase) never gates PE; out-proj for chunk c
                # goes in once all 4 of its norms have flushed
                sched = [(0, 0), (1, 0), (0, 1), (1, 1), (2, 0), (3, 0),
                         (2, 1), (3, 1), "o0", (0, 2), (1, 2), "o1",
                         (2, 2), (3, 2), (0, 3), (1, 3), "o2", (2, 3), (3, 3)]
                for item in sched:
                    if isinstance(item, str):
                        outproj_chunk(int(item[1]), psps, pout)
                    else:
                        attn_chunk(item[0], item[1], psps, psps)
                norm_chunk(*norm_state["pending"], psps, psps)
                outproj_chunk(3, psps, pout)

            pmid.release()
            pnm.release()
            pat.release()
    nc.compile()
    _NC_CACHE[("nc", has_vbias)] = nc
    return nc


def make_in_maps(x, cumulative_scores, padding_mask, W_attn, b_attn, W_proj,
                 b_proj, token_index):
    x = np.asarray(x, dtype=np.float32)
    cs = np.asarray(cumulative_scores, dtype=np.float32)
    pm = np.asarray(padding_mask, dtype=np.float32)
    Wa = np.asarray(W_attn, dtype=np.float32)
    ba = np.asarray(b_attn, dtype=np.float32)
    Wp = np.asarray(W_proj, dtype=np.float32)
    tok = np.asarray(token_index).astype(np.int64)
    B = x.shape[0]

    # single upper-triangular 0/1 mask for diagonal blocks: m[p,c] = c >= p
    dmask = (np.arange(P)[None, :] >= np.arange(P)[:, None]).astype(BF)
    ones_row = np.ones((1, T), BF)
    ones_r = np.ones((1, 64), np.float32)
    # signed rotate-half permutation as matmul lhsT: rsel[p, d] = s(d) where
    # p = rot(d); fork columns 63/127 zeroed (fork rows come from fsel)
    rsel = np.zeros((P, P), np.float32)
    for blk in (0, 64):
        for j in range(32):
            rsel[blk + j + 32, blk + j] = -1.0      # dst j < 32: -x[j+32]
        for j in range(32, 63):
            rsel[blk + j - 32, blk + j] = 1.0       # dst j >= 32: +x[j-32]
        rsel[blk + 31, blk + 63] = 0.0
    fsel = np.zeros((1, P), np.float32)
    fsel[0, 63] = 1.0
    fsel[0, 127] = 1.0

    per_batch = []
    for b in range(B):
        counts = np.bincount(tok[b], minlength=T).astype(np.float32)
        with np.errstate(divide="ignore"):
            invc = (1.0 / counts).astype(np.float32)
        partial = np.cumsum(invc[tok[b]], dtype=np.float32)
        invf = (1.0 / (10000.0 ** (np.arange(0, 64, 2, dtype=np.float32) / 64.0))
                ).astype(np.float32)
        ang = partial[:, None].astype(np.float32) * invf[None, :]
        cos32 = np.cos(ang).T.astype(np.float32)
        sin32 = np.sin(ang).T.astype(np.float32)
        # fork rows folded into the tables: cos rows 63/127 -> 0 and sin
        # rows -> 1, so rope emits the fork values injected by the fsel matmul
        cos128 = np.tile(cos32, (4, 1))
        cos128[63, :] = 0.0
        cos128[127, :] = 0.0
        sin128 = np.tile(sin32, (4, 1))
        sin128[63, :] = 1.0
        sin128[127, :] = 1.0
        pmg = np.take_along_axis(pm[b], tok[b], axis=0).astype(np.float32)
        pmbin = (pmg != 0).astype(np.float32)
        vscale = (np.exp(cs[b]).astype(np.float32) * pmbin).astype(np.float32)
        per_batch.append({
            "xt": np.ascontiguousarray(x[b].T).astype(BF),
            "cos2": np.ascontiguousarray(cos128).astype(BF),
            "sintab": np.ascontiguousarray(sin128).astype(BF),
            "csrow": np.ascontiguousarray(cs[b][None, :]).astype(BF),
            "vscale": np.ascontiguousarray(vscale.reshape(NT, P).T),
            "onec": np.ascontiguousarray(pmbin.reshape(NT, P).T).astype(BF),
        })

    in_maps = []
    for core in range(8):
        b = core // 4
        g = core % 4
        qc = slice(g * 256, (g + 1) * 256)
        kc = slice(C + g * 256, C + (g + 1) * 256)
        vc = slice(2 * C + g * 256, 2 * C + (g + 1) * 256)
        wqk = np.ascontiguousarray(
            np.concatenate([Wa[:, qc], Wa[:, kc]], axis=1)).astype(BF)
        bqk_flat = np.concatenate([ba[qc], ba[kc]])          # [512]
        bqk = np.ascontiguousarray(bqk_flat.reshape(4, P).T)
        in_maps.append({
            **per_batch[b],
            "wqk": wqk,
            "wv": np.ascontiguousarray(Wa[:, vc]).astype(BF),
            "wp": np.ascontiguousarray(Wp[g * 256:(g + 1) * 256, :]).astype(BF),
            "bqk": bqk,
            "bv": np.ascontiguousarray(ba[vc][None, :]).astype(BF),
            "dmask": dmask,
            "ones": ones_row,
            "onesr": ones_r,
            "rsel": rsel.astype(BF),
            "fsel": fsel.astype(BF),
        })
    return in_maps


def kernel(x, cumulative_scores, padding_mask, W_attn, b_attn, W_proj, b_proj,
           token_index, _results_hook=None):
    nc = build_nc(has_vbias=bool(np.any(np.asarray(b_attn)[2 * C:])))
    in_maps = make_in_maps(x, cumulative_scores, padding_mask, W_attn, b_attn,
                           W_proj, b_proj, token_index)
    res = run_bass_kernel_spmd(nc, in_maps, list(range(8)))
    if _results_hook is not None:
        _results_hook(res)
    bp = np.asarray(b_proj, dtype=np.float32)
    B = np.asarray(x).shape[0]
    out = np.zeros((B, T, C), np.float32)
    for b in range(B):
        acc = np.zeros((T, C), np.float32)
        for g in range(4):
            acc += np.asarray(res.results[b * 4 + g]["outp"]).astype(np.float32)
        out[b] = acc + bp[None, :]
    return out


# revision 38
# speedup vs baseline: 1.0172x; 1.0001x over previous
"""Trainium2 Bass kernel for nn_CausalSelfAttention_67894843015857.

Full inputs -> full output. Sharding: 8 cores = 2 batches x 4 head-groups
(4 heads each). Per core, on device (all matmul operands bf16, PSUM f32):
  - q,k projections (W stationary, x^T moving) -> qT/kT in [dim, t] layout
  - RoPE (host-precomputed cos/sin tables from token_index histogram cumsum)
    done full-width per head-pair via one DMA-built partition-rotated copy,
    then fork-channel row overwrites
  - v projection (x^T stationary) -> V in [t, dim] layout, scaled by
    exp(cumulative_scores)*padmask, with a ones-column appended (softmax denom)
  - attention with TRANSPOSED scores S^T[tk, tq] (no P-transpose needed),
    no-max-subtraction softmax (scores bounded ~2.6), causal masking via
    0/1 masks on the 4 diagonal blocks of each 512-wide tq chunk.
    Chunk-major over tq so the output projection for chunk c-1 interleaves
    into chunk c's attention (overlaps the out DMA with compute).
  - output projection -> per-core partial [T, C] stored bf16
Host: reduces the 4 per-batch partials and adds b_proj.
"""
import numpy as np
import ml_dtypes

import concourse.bacc as bacc
import concourse.mybir as mybir
import concourse.tile as tile
from concourse.bass_utils import run_bass_kernel_spmd

F32 = mybir.dt.float32
F32R = mybir.dt.float32r
BF16 = mybir.dt.bfloat16
AF = mybir.ActivationFunctionType
BF = ml_dtypes.bfloat16

P = 128
T = 2048
C = 1024
NKT = C // P          # 8 contraction tiles over the embedding dim
NT = T // P           # 16 t-tiles
SCALE = 0.125         # 1/sqrt(64)
NCHUNK = 4            # tq chunks of 512
CH = 512

_NC_CACHE = {}


def build_nc(has_vbias=False):
    if ("nc", has_vbias) in _NC_CACHE:
        return _NC_CACHE[("nc", has_vbias)]
    nc = bacc.Bacc("TRN2", target_bir_lowering=False, debug=False)

    def din(name, shape, dt=BF16):
        return nc.dram_tensor(name, shape, dt, kind="ExternalInput").ap()

    xt_d = din("xt", [C, T])            # x[b].T
    wqk_d = din("wqk", [C, 512])        # [q cols 256 | k cols 256] for this head group
    wv_d = din("wv", [C, 256])
    wp_d = din("wp", [256, C])
    bqk_d = din("bqk", [P, 4], F32)     # bias per col-tile m
    bv_d = din("bv", [1, 256])
    cos_d = din("cos2", [P, T])         # cos table (rows 63,127 zeroed)
    sin_d = din("sintab", [P, T])       # +sin table (rows 63,127 = 1.0)
    rsel_d = din("rsel", [P, P])        # signed rotate-half permutation (lhsT)
    fsel_d = din("fsel", [1, P])        # one-hot fork-row selector (lhsT)
    cs_d = din("csrow", [1, T])         # cumulative_scores[b] (fork-k rows)
    vs_d = din("vscale", [P, 16], F32)  # exp(cs)*pmbin, t-tiled columns
    oc_d = din("onec", [P, 16])         # pmbin, t-tiled columns
    dm_d = din("dmask", [P, P])         # upper-tri 0/1 mask: m[p,c] = c >= p
    ones_d = din("ones", [1, T])        # fork-q rows + bf16 bias-matmul lhsT
    onr_d = din("onesr", [1, 64], F32R)  # f32r lhsT for the norm broadcast
    out_d = nc.dram_tensor("outp", [T, C], BF16, kind="ExternalOutput").ap()

    with tile.TileContext(nc) as tc:
        with tc.tile_pool(name="const", bufs=1) as pc, \
             tc.tile_pool(name="persist", bufs=1) as pp:
            bqk_sb = pc.tile([P, 4], F32, name="bqk_sb")
            bv_sb = pc.tile([1, 256], BF16, name="bv_sb")
            ones_sb = pc.tile([1, T], BF16, name="ones_sb")
            onr_sb = pc.tile([1, 64], F32R, name="onr_sb")
            cs_sb = pc.tile([1, T], BF16, name="cs_sb")
            vs_sb = pc.tile([P, 16], F32, name="vs_sb")
            oc_sb = pc.tile([P, 16], BF16, name="oc_sb")
            dm_sb = pc.tile([P, P], BF16, name="dm_sb")
            rsel_sb = pc.tile([P, P], BF16, name="rsel_sb")
            fsel_sb = pc.tile([1, P], BF16, name="fsel_sb")

            qk_t = [pp.tile([P, T], BF16, name=f"qkt{m}") for m in range(4)]
            vv = pp.tile([P, NT * 260], BF16, name="vv")
            yt = [pp.tile([P, T], BF16, name=f"yt{i}") for i in range(2)]
            wp_sb = pp.tile([P, 2 * C], BF16, name="wp_sb")

            pat = tc.alloc_tile_pool(name="attn_sb", bufs=5)
            pnm = tc.alloc_tile_pool(name="norm_sb", bufs=2)
            pmid = tc.alloc_tile_pool(name="mid", bufs=1)
            with tc.tile_pool(name="load", bufs=1) as pl:
                xt = pl.tile([P, NKT * T], BF16, name="xt_sb")
                wqk = pl.tile([P, NKT * 512], BF16, name="wqk_sb")
                # k=0 weights + x chunks first (earliest matmul start)
                nc.sync.dma_start(wqk[:, 0:512], wqk_d[0:P, :])
                for n in range(4):
                    nc.sync.dma_start(xt[:, n * CH:(n + 1) * CH],
                                      xt_d[0:P, n * CH:(n + 1) * CH])
                for k in range(1, NKT):
                    nc.sync.dma_start(wqk[:, k * 512:(k + 1) * 512],
                                      wqk_d[k * P:(k + 1) * P, :])
                    nc.sync.dma_start(xt[:, k * T:(k + 1) * T],
                                      xt_d[k * P:(k + 1) * P, :])
                cos_sb = pmid.tile([P, T], BF16, name="cos_sb")
                nc.sync.dma_start(cos_sb[:], cos_d[:])
                sin_sb = pmid.tile([P, T], BF16, name="sin_sb")
                nc.sync.dma_start(sin_sb[:], sin_d[:])
                nc.sync.dma_start(rsel_sb[:], rsel_d[:])
                nc.sync.dma_start(fsel_sb[:], fsel_d[:])
                nc.sync.dma_start(bqk_sb[:], bqk_d[:])
                nc.sync.dma_start(bv_sb[:], bv_d[:])
                nc.sync.dma_start(ones_sb[:], ones_d[:])
                nc.sync.dma_start(onr_sb[:], onr_d[:])
                nc.sync.dma_start(cs_sb[:], cs_d[:])
                nc.sync.dma_start(vs_sb[:], vs_d[:])
                nc.sync.dma_start(oc_sb[:], oc_d[:])
                nc.sync.dma_start(dm_sb[:], dm_d[:])
                wv = pl.tile([P, NKT * 256], BF16, name="wv_sb")
                nc.sync.dma_start(
                    wv[:].rearrange("p (k c) -> p k c", k=NKT),
                    wv_d[:].rearrange("(k p) c -> p k c", k=NKT))
                nc.sync.dma_start(
                    wp_sb[:].rearrange("p (k c) -> p k c", k=2),
                    wp_d[:].rearrange("(k p) c -> p k c", k=2))

                def qk_mm(ms, qkps, tagbase=0):
                    """q,k projections for col-tiles `ms`, k-outer so PE rides
                    the incoming xt DMA stream; then psum->sbuf bias copies."""
                    pss = {(m, n): qkps.tile([P, CH], F32, name=f"qkps{m}_{n}",
                                             tag=f"a{tagbase + 4 * ms.index(m) + n}")
                           for m in ms for n in range(4)}
                    for k in range(NKT):
                        for m in ms:
                            for n in range(4):
                                nc.tensor.matmul(
                                    pss[m, n][:],
                                    lhsT=wqk[:, k * 512 + m * P: k * 512 + (m + 1) * P],
                                    rhs=xt[:, k * T + n * CH: k * T + (n + 1) * CH],
                                    start=(k == 0), stop=(k == NKT - 1))
                    # psum->sbuf copies split ACT/DVE so the bridge into the
                    # next PSUM users drains twice as fast
                    for n in range(4):
                        for m in ms:
                            ns = slice(n * CH, (n + 1) * CH)
                            if (n + ms.index(m)) % 2 == 0:
                                nc.scalar.activation(qk_t[m][:, ns], pss[m, n][:],
                                                     AF.Identity,
                                                     bias=bqk_sb[:, m:m + 1])
                            else:
                                nc.vector.tensor_scalar_add(
                                    qk_t[m][:, ns], pss[m, n][:],
                                    bqk_sb[:, m:m + 1])

                def rope_group(m, ps, tagbase):
                    """Chunked in-place RoPE on qk_t[m]: the rotate-half copy
                    comes from a signed-permutation matmul (plus a K=1 matmul
                    injecting the fork rows: cos rows 63,127 are zeroed and
                    sin rows are 1.0 there, so rope yields the fork values)."""
                    src = ones_sb if m < 2 else cs_sb
                    for n in range(4):
                        ns = slice(n * CH, (n + 1) * CH)
                        qs = ps.tile([P, CH], F32, name=f"qs{m}_{n}",
                                     tag=f"a{tagbase + n}")
                        nc.tensor.matmul(qs[:], lhsT=rsel_sb[:],
                                         rhs=qk_t[m][:, ns],
                                         start=True, stop=False)
                        nc.tensor.matmul(qs[:], lhsT=fsel_sb[0:1, :],
                                         rhs=src[0:1, ns],
                                         start=False, stop=True)
                        qsh = pmid.tile([P, CH], BF16, name=f"qsh{m}_{n}",
                                        tag="qsh", bufs=2)
                        nc.vector.tensor_mul(qk_t[m][:, ns], qk_t[m][:, ns],
                                             cos_sb[:, ns])
                        nc.vector.tensor_mul(qsh[:], qs[:], sin_sb[:, ns])
                        nc.vector.tensor_add(qk_t[m][:, ns], qk_t[m][:, ns],
                                             qsh[:])

                def v_group(mt0, mt1, vpsp):
                    """v projection: out[t, vcol] = xT_tile.T @ wv; build V''."""
                    for mt in range(mt0, mt1):
                        vps = vpsp.tile([P, 256], F32, name=f"vps{mt}",
                                        tag=f"a{mt % 4}")
                        for k in range(NKT):
                            nc.tensor.matmul(
                                vps[:],
                                lhsT=xt[:, k * T + mt * P: k * T + (mt + 1) * P],
                                rhs=wv[:, k * 256:(k + 1) * 256],
                                start=(k == 0),
                                stop=(not has_vbias and k == NKT - 1))
                        if has_vbias:
                            nc.tensor.matmul(vps[:], lhsT=ones_sb[0:1, 0:P],
                                             rhs=bv_sb[0:1, :], start=False,
                                             stop=True)
                        vvs = vv[:, mt * 260:(mt + 1) * 260].rearrange(
                            "p (h x) -> p h x", x=65)
                        nc.vector.tensor_scalar_mul(
                            vvs[:, :, 0:64],
                            vps[:].rearrange("p (h x) -> p h x", x=64),
                            vs_sb[:, mt:mt + 1])
                        nc.vector.tensor_copy(
                            vvs[:, :, 64:65],
                            oc_sb[:, mt:mt + 1, None].to_broadcast((P, 4, 1)))

                with nc.named_scope("proj"), \
                     tc.tile_pool(name="ps1", bufs=1, space="PSUM") as ps1:
                    qk_mm((0, 2), ps1)     # q heads 0,1 + k heads 0,1
                    rope_group(0, ps1, 0)
                    rope_group(2, ps1, 4)
                    v_group(0, 4, ps1)
                    qk_mm((1,), ps1, 4)    # q heads 2,3
                    rope_group(1, ps1, 4)
                    qk_mm((3,), ps1, 4)    # k heads 2,3
                    rope_group(3, ps1, 4)
                    v_group(4, 16, ps1)

            # load pool released: xt/wqk/wv space reusable
            def norm_chunk(h, cch, yps, pyps, psps):
                # normalize: y = num / den  (den = ones-column row 64)
                ti = h // 2
                ro = 64 * (h % 2)
                recip = pnm.tile([1, CH], F32R, name=f"rc_{h}_{cch}", tag="rc")
                with nc.allow_low_precision(reason="f32r recip of f32 denom"):
                    nc.vector.reciprocal(recip[0:1, :], yps[64:65, :])
                # broadcast along partitions on the (otherwise idle) GPSIMD
                bsb = pnm.tile([64, CH], F32R, name=f"bs_{h}_{cch}", tag="bs")
                nc.gpsimd.partition_broadcast(bsb[:], recip[0:1, :])
                nc.vector.tensor_mul(
                    yt[ti][ro:ro + 64, cch * CH:(cch + 1) * CH],
                    yps[0:64, :], bsb[:])

            norm_state = {"pending": None}

            def attn_chunk(h, cch, psps, pyps):
                ti = h // 2
                ro = 64 * (h % 2)
                qt = qk_t[ti]
                kt = qk_t[2 + ti]
                nik = 4 * (cch + 1)
                yps = pyps.tile([65, CH], F32, name=f"yps_{h}_{cch}", tag="yps")
                for p2 in range(nik // 2):
                    # an ik pair shares one 2-bank PSUM tile so full
                    # pairs need only a single wide exp
                    spw = psps.tile([P, 2 * CH], F32,
                                    name=f"spw_{h}_{cch}_{p2}", tag="sps")
                    pt = pat.tile([P, 2 * CH], BF16,
                                  name=f"pt_{h}_{cch}_{p2}", tag="pt")
                    iks = (2 * p2, 2 * p2 + 1)
                    los = [max(ik - 4 * cch, 0) * P for ik in iks]
                    for ii, ik in enumerate(iks):
                        lo = los[ii]
                        nc.tensor.matmul(
                            spw[:, ii * CH + lo:(ii + 1) * CH],
                            lhsT=kt[ro:ro + 64, ik * P:(ik + 1) * P],
                            rhs=qt[ro:ro + 64, cch * CH + lo:(cch + 1) * CH],
                            start=True, stop=True)
                    if los[0] == 0 and los[1] == 0:
                        nc.scalar.activation(pt[:], spw[:], AF.Exp,
                                             scale=SCALE)
                    else:
                        for ii, ik in enumerate(iks):
                            lo = los[ii]
                            nc.scalar.activation(
                                pt[:, ii * CH + lo:(ii + 1) * CH],
                                spw[:, ii * CH + lo:(ii + 1) * CH],
                                AF.Exp, scale=SCALE)
                    for ii, ik in enumerate(iks):
                        lo = los[ii]
                        if ik - 4 * cch >= 0:
                            # triangular mask on the diagonal block
                            nc.vector.tensor_mul(
                                pt[:, ii * CH + lo: ii * CH + lo + P],
                                pt[:, ii * CH + lo: ii * CH + lo + P],
                                dm_sb[:])
                        nc.tensor.matmul(
                            yps[:, lo:CH],
                            lhsT=vv[:, ik * 260 + h * 65: ik * 260 + h * 65 + 65],
                            rhs=pt[:, ii * CH + lo:(ii + 1) * CH],
                            start=(ik == 0), stop=(ik == nik - 1))
                    if p2 == 0 and norm_state["pending"] is not None:
                        # previous chunk's norm, deep in this chunk's
                        # pipeline so it never stalls PE/ACT
                        norm_chunk(*norm_state["pending"], pyps, psps)
                        norm_state["pending"] = None
                norm_state["pending"] = (h, cch, yps)

            def outproj_chunk(cch, psps, pout):
                for mt in range(4 * cch, 4 * cch + 4):
                    osb = pout.tile([P, C], BF16, name=f"osb{mt}", tag="osb")
                    for n in range(2):
                        pps = psps.tile([P, CH], F32, name=f"pps{mt}_{n}",
                                        tag="scr")
                        for kk in range(2):
                            nc.tensor.matmul(
                                pps[:],
                                lhsT=yt[kk][:, mt * P:(mt + 1) * P],
                                rhs=wp_sb[:, kk * C + n * CH: kk * C + (n + 1) * CH],
                                start=(kk == 0), stop=(kk == 1))
                        if n == 0:
                            nc.scalar.copy(osb[:, 0:CH], pps[:])
                        else:
                            nc.vector.tensor_copy(osb[:, CH:C], pps[:])
                    nc.sync.dma_start(out_d[mt * P:(mt + 1) * P, :], osb[:])

            with nc.named_scope("attn"), \
                 tc.tile_pool(name="ps2", bufs=2, space="PSUM") as psps, \
                 tc.tile_pool(name="out_sb", bufs=3) as pout:
                # h2/h3 chunks deferred so their RoPE (critical path out of
                # the projection phase) never gates PE; out-proj for chunk c
                # goes in once all 4 of its norms have flushed
                sched = [(0, 0), (1, 0), (0, 1), (1, 1), (2, 0), (3, 0),
                         (2, 1), (3, 1), "o0", (0, 2), (1, 2), "o1",
                         (2, 2), (3, 2), (0, 3), (1, 3), "o2", (2, 3), (3, 3)]
                for item in sched:
                    if isinstance(item, str):
                        outproj_chunk(int(item[1]), psps, pout)
                    else:
                        attn_chunk(item[0], item[1], psps, psps)
                norm_chunk(*norm_state["pending"], psps, psps)
                outproj_chunk(3, psps, pout)

            pmid.release()
            pnm.release()
            pat.release()
    nc.compile()
    _NC_CACHE[("nc", has_vbias)] = nc
    return nc


def make_in_maps(x, cumulative_scores, padding_mask, W_attn, b_attn, W_proj,
                 b_proj, token_index):
    x = np.asarray(x, dtype=np.float32)
    cs = np.asarray(cumulative_scores, dtype=np.float32)
    pm = np.asarray(padding_mask, dtype=np.float32)
    Wa = np.asarray(W_attn, dtype=np.float32)
    ba = np.asarray(b_attn, dtype=np.float32)
    Wp = np.asarray(W_proj, dtype=np.float32)
    tok = np.asarray(token_index).astype(np.int64)
    B = x.shape[0]

    # single upper-triangular 0/1 mask for diagonal blocks: m[p,c] = c >= p
    dmask = (np.arange(P)[None, :] >= np.arange(P)[:, None]).astype(BF)
    ones_row = np.ones((1, T), BF)
    ones_r = np.ones((1, 64), np.float32)
    # signed rotate-half permutation as matmul lhsT: rsel[p, d] = s(d) where
    # p = rot(d); fork columns 63/127 zeroed (fork rows come from fsel)
    rsel = np.zeros((P, P), np.float32)
    for blk in (0, 64):
        for j in range(32):
            rsel[blk + j + 32, blk + j] = -1.0      # dst j < 32: -x[j+32]
        for j in range(32, 63):
            rsel[blk + j - 32, blk + j] = 1.0       # dst j >= 32: +x[j-32]
        rsel[blk + 31, blk + 63] = 0.0
    fsel = np.zeros((1, P), np.float32)
    fsel[0, 63] = 1.0
    fsel[0, 127] = 1.0

    per_batch = []
    for b in range(B):
        counts = np.bincount(tok[b], minlength=T).astype(np.float32)
        with np.errstate(divide="ignore"):
            invc = (1.0 / counts).astype(np.float32)
        partial = np.cumsum(invc[tok[b]], dtype=np.float32)
        invf = (1.0 / (10000.0 ** (np.arange(0, 64, 2, dtype=np.float32) / 64.0))
                ).astype(np.float32)
        ang = partial[:, None].astype(np.float32) * invf[None, :]
        cos32 = np.cos(ang).T.astype(np.float32)
        sin32 = np.sin(ang).T.astype(np.float32)
        # fork rows folded into the tables: cos rows 63/127 -> 0 and sin
        # rows -> 1, so rope emits the fork values injected by the fsel matmul
        cos128 = np.tile(cos32, (4, 1))
        cos128[63, :] = 0.0
        cos128[127, :] = 0.0
        sin128 = np.tile(sin32, (4, 1))
        sin128[63, :] = 1.0
        sin128[127, :] = 1.0
        pmg = np.take_along_axis(pm[b], tok[b], axis=0).astype(np.float32)
        pmbin = (pmg != 0).astype(np.float32)
        vscale = (np.exp(cs[b]).astype(np.float32) * pmbin).astype(np.float32)
        per_batch.append({
            "xt": np.ascontiguousarray(x[b].T).astype(BF),
            "cos2": np.ascontiguousarray(cos128).astype(BF),
            "sintab": np.ascontiguousarray(sin128).astype(BF),
            "csrow": np.ascontiguousarray(cs[b][None, :]).astype(BF),
            "vscale": np.ascontiguousarray(vscale.reshape(NT, P).T),
            "onec": np.ascontiguousarray(pmbin.reshape(NT, P).T).astype(BF),
        })

    in_maps = []
    for core in range(8):
        b = core // 4
        g = core % 4
        qc = slice(g * 256, (g + 1) * 256)
        kc = slice(C + g * 256, C + (g + 1) * 256)
        vc = slice(2 * C + g * 256, 2 * C + (g + 1) * 256)
        wqk = np.ascontiguousarray(
            np.concatenate([Wa[:, qc], Wa[:, kc]], axis=1)).astype(BF)
        bqk_flat = np.concatenate([ba[qc], ba[kc]])          # [512]
        bqk = np.ascontiguousarray(bqk_flat.reshape(4, P).T)
        in_maps.append({
            **per_batch[b],
            "wqk": wqk,
            "wv": np.ascontiguousarray(Wa[:, vc]).astype(BF),
            "wp": np.ascontiguousarray(Wp[g * 256:(g + 1) * 256, :]).astype(BF),
            "bqk": bqk,
            "bv": np.ascontiguousarray(ba[vc][None, :]).astype(BF),
            "dmask": dmask,
            "ones": ones_row,
            "onesr": ones_r,
            "rsel": rsel.astype(BF),
            "fsel": fsel.astype(BF),
        })
    return in_maps


def kernel(x, cumulative_scores, padding_mask, W_attn, b_attn, W_proj, b_proj,
           token_index, _results_hook=None):
    nc = build_nc(has_vbias=bool(np.any(np.asarray(b_attn)[2 * C:])))
    in_maps = make_in_maps(x, cumulative_scores, padding_mask, W_attn, b_attn,
                           W_proj, b_proj, token_index)
    res = run_bass_kernel_spmd(nc, in_maps, list(range(8)))
    if _results_hook is not None:
        _results_hook(res)
    bp = np.asarray(b_proj, dtype=np.float32)
    B = np.asarray(x).shape[0]
    out = np.zeros((B, T, C), np.float32)
    for b in range(B):
        acc = np.zeros((T, C), np.float32)
        for g in range(4):
            acc += np.asarray(res.results[b * 4 + g]["outp"]).astype(np.float32)
        out[b] = acc + bp[None, :]
    return out
